# revision 1
# baseline (speedup 1.0000x reference)
"""AntiAliasInterpolation2d Trainium kernel.

out[n,i,j,c] = sum_{dy,dx} g[dy]*g[dx] * x[n, 4i+dy-6, 4j+dx-6, c]   (zero pad)

i.e. a separable 13-tap Gaussian blur evaluated only on the stride-4 output
grid (the nearest-neighbor downsample of the reference picks blurred[4i,4j]).

Per core (batch shard of 4 images):
  vertical:   t1[i, w]  = sum_h AB[h, i] * x[h, w]     (TensorE matmul,
              AB is the banded 512x128 matrix AB[h,i] = g[h-4i+6])
  horizontal: out[i, j] = sum_dx g[dx] * t1[i, 4j+dx-6] (13 strided MACs on DVE)

Built on bacc.Bacc: its generate_event_semaphores pass splits Tile's
multi-semaphore waits into EventSemaphore instructions (this walrus build
allows at most one semaphore wait per regular instruction).
"""

import numpy as np

try:
    import concourse.bass as bass
except ImportError:  # pragma: no cover
    import sys

    sys.path.insert(0, "/opt/trn_rl_repo")
    import concourse.bass as bass

import concourse.mybir as mybir
from concourse import bacc, tile
from concourse.bass_utils import run_bass_kernel_spmd

N_CORES = 8
N_PER_CORE = 4          # 32 images / 8 cores
H = W = 512
C = 3
OH = OW = 128
KSIZE = 13
KA = 6
SIGMA = 1.5


def _gauss_norm() -> np.ndarray:
    r = np.arange(KSIZE, dtype=np.float32)
    g = np.exp(-((r - np.float32(KA)) ** 2) / np.float32(2.0 * SIGMA * SIGMA))
    return (g / g.sum()).astype(np.float32)


def _band_matrix() -> np.ndarray:
    """AB[h, i] = g[h - 4i + 6], zero outside the band."""
    g = _gauss_norm()
    ab = np.zeros((H, OH), dtype=np.float32)
    for i in range(OH):
        for dy in range(KSIZE):
            h = 4 * i + dy - KA
            if 0 <= h < H:
                ab[h, i] = g[dy]
    return ab


def _tap_ranges():
    """For each dx: (j0, j1, r, q) s.t. src w-index = 4*(j+q) + r for j in [j0,j1)."""
    taps = []
    for dx in range(KSIZE):
        off = dx - KA
        j0 = 0 if off >= 0 else (-off + 3) // 4  # ceil(-off/4)
        j1 = min(OW, (W - 1 - off) // 4 + 1)
        r = off % 4
        q = (off - r) // 4
        taps.append((dx, j0, j1, r, q))
    return taps


def build_nc(
    repeats: int = 1,
    n_chunks: int = 4,
    sbuf_taps: int = 0,
    dma_only: int = 0,
    tail_split: int = 0,
) -> bass.Bass:
    """repeats>1 re-runs the whole per-core program (for timing benchmarks).
    n_chunks: x DMAs per image (1, 2, or 4 h-blocks per DMA).
    sbuf_taps: copy t1 PSUM->SBUF on ACT first (measured slower on HW).
    dma_only: benchmark variant that skips all compute.
    tail_split: stream the LAST image in bank-aligned W-thirds so most of
      its tap work overlaps the final DMAs. Off by default: the DVE is
      still draining image 2's taps when image 3's early thirds land, so
      the split only adds op overhead (cost model: 52.5us vs 51.3us)."""
    nc = bacc.Bacc()
    f32 = mybir.dt.float32
    # float32r: same 32-bit storage, but the PE streams it at 1 cycle/row
    # (plain float32 matmuls decompose into 2 half-rate passes = 4x slower)
    f32r = mybir.dt.float32r
    x = nc.declare_dram_parameter("x", [N_PER_CORE, H, W, C], f32r, isOutput=False)
    ab = nc.declare_dram_parameter("ab", [H, OH], f32r, isOutput=False)
    out = nc.declare_dram_parameter("out", [N_PER_CORE, OH, OW, C], f32, isOutput=True)

    g = _gauss_norm()
    taps = _tap_ranges()
    # full-coverage tap (dx=6) first so it can initialize the accumulator
    taps.sort(key=lambda t: t[0] != KA)

    with tile.TileContext(nc) as tc:
        with (
            tc.tile_pool(name="const", bufs=1) as cpool,
            tc.tile_pool(name="xp", bufs=1) as xpool,
            tc.tile_pool(name="op", bufs=1) as opool,
            tc.tile_pool(name="ps", bufs=2, space="PSUM") as pspool,
        ):
            # banded vertical matrix: sbuf [p=h%128, (k, i)] from dram
            # [(k p), i]; issued on the ACT HWDGE queue so it doesn't delay
            # the first x chunk at the head of the SP queue
            ab_s = cpool.tile([128, 4 * OH], f32r)
            nc.scalar.dma_start(
                out=ab_s[:].rearrange("p (k i) -> p k i", k=4),
                in_=ab.rearrange("(k p) i -> p k i", p=128),
            )

            kb = 4 // n_chunks  # h-blocks per DMA

            def emit_image(n):
                # per-chunk DMAs: matmuls for a chunk start as soon as it
                # lands instead of waiting for the whole 3MB image;
                # dedicated tiles so the DMAs have no WAR deps
                xts = []
                for ck in range(n_chunks):
                    xtk = xpool.tile(
                        [128, kb * W * C], f32r, tag=f"xt{n}k{ck}", name=f"xt{n}k{ck}"
                    )
                    nc.sync.dma_start(
                        out=xtk[:].rearrange("p (b f) -> p b f", b=kb),
                        in_=x[n].rearrange("(ck b p) w c -> ck p b (w c)", p=128, b=kb)[
                            ck
                        ],
                    )
                    xts.append(xtk)

                if dma_only:
                    ot = opool.tile([128, OW * C], f32, tag=f"ot{n}", name=f"ot{n}")
                    nc.vector.tensor_copy(ot[:], xts[0][:, : OW * C])
                    nc.scalar.dma_start(
                        out=out[n].rearrange("i j c -> i (j c)"), in_=ot[:]
                    )
                    return

                # vertical blur via matmul, on the INTERLEAVED (w c) layout:
                # every column of x is blurred independently, so rhs can be
                # contiguous 512-element slices (PE streams at line rate;
                # strided rhs would throttle the XBUS). t1 free index is
                # m = w*3 + c.
                t1 = pspool.tile([128, C * W], f32, tag="t1", name=f"t1_{n}")
                for k in range(4):
                    lhsT = ab_s[:, k * OH : (k + 1) * OH]
                    xvk = xts[k // kb][:].rearrange("p (b f) -> p b f", b=kb)[
                        :, k % kb
                    ]
                    for s in range(C):
                        nc.tensor.matmul(
                            t1[:, s * W : (s + 1) * W],
                            lhsT,
                            xvk[:, s * W : (s + 1) * W],
                            start=(k == 0),
                            stop=(k == 3),
                        )

                if sbuf_taps:
                    # PSUM -> SBUF via ACT so the DVE taps run all-SBUF
                    # (2x_2p mode in the cost model; measured slower on HW)
                    t1s = opool.tile(
                        [128, C * W], f32, tag=f"t1s{n}", name=f"t1s{n}"
                    )
                    nc.scalar.copy(t1s[:], t1[:])
                    tap_src = t1s
                else:
                    tap_src = t1

                # horizontal blur: 13 strided MACs on DVE
                # src index m = w*3 + c with w = 4u + r -> view [p, r, u, c]
                # (c innermost: each AP step covers a contiguous 12B triple)
                t1v = tap_src[:].rearrange("p (u r c) -> p r u c", r=4, c=C)
                ot = opool.tile([128, OW * C], f32, tag=f"ot{n}", name=f"ot{n}")
                ov = ot[:].rearrange("p (j c) -> p j c", c=C)

                first = True
                for dx, j0, j1, r, q in taps:
                    src = t1v[:, r, j0 + q : j1 + q]
                    dst = ov[:, j0:j1]
                    if first:
                        first = False
                        nc.vector.tensor_scalar(
                            dst, src, float(g[dx]), None, mybir.AluOpType.mult
                        )
                    else:
                        nc.vector.scalar_tensor_tensor(
                            dst,
                            src,
                            float(g[dx]),
                            dst,
                            mybir.AluOpType.mult,
                            mybir.AluOpType.add,
                        )

                # out DMA on the ACT HWDGE queue: its wait on the taps must
                # not block dispatch of later x DMAs on the SP queue
                nc.scalar.dma_start(
                    out=out[n].rearrange("i j c -> i (j c)"), in_=ot[:]
                )

            def emit_image_tailsplit(n):
                # Last image of the stream: DMA it in 12 bank-aligned
                # W-thirds (third-major), and run the taps in two phases.
                # Phase A (j < 84) reads only PSUM banks 0-1 (m <= 1019)
                # so it overlaps the final third's DMAs + matmuls; only
                # phase B (j >= 84, ~1/3 of the tap work) trails the last
                # byte.
                JB = 84
                xts = {}
                for s in range(C):
                    for k in range(4):
                        t = xpool.tile(
                            [128, W], f32r, tag=f"xs{n}s{s}k{k}", name=f"xs{n}s{s}k{k}"
                        )
                        nc.sync.dma_start(
                            out=t[:],
                            in_=x[n].rearrange("(k p) w c -> k p (w c)", p=128)[k][
                                :, 512 * s : 512 * (s + 1)
                            ],
                        )
                        xts[(s, k)] = t

                t1 = pspool.tile([128, C * W], f32, tag="t1", name=f"t1_{n}")
                t1v = t1[:].rearrange("p (u r c) -> p r u c", r=4, c=C)
                ot = opool.tile([128, OW * C], f32, tag=f"ot{n}", name=f"ot{n}")
                ov = ot[:].rearrange("p (j c) -> p j c", c=C)

                def emit_taps(jlo, jhi):
                    first = True
                    for dx, j0, j1, r, q in taps:
                        jl, jh = max(j0, jlo), min(j1, jhi)
                        if jl >= jh:
                            continue
                        src = t1v[:, r, jl + q : jh + q]
                        dst = ov[:, jl:jh]
                        if first:
                            first = False
                            nc.vector.tensor_scalar(
                                dst, src, float(g[dx]), None, mybir.AluOpType.mult
                            )
                        else:
                            nc.vector.scalar_tensor_tensor(
                                dst,
                                src,
                                float(g[dx]),
                                dst,
                                mybir.AluOpType.mult,
                                mybir.AluOpType.add,
                            )

                for s in range(C):
                    for k in range(4):
                        nc.tensor.matmul(
                            t1[:, 512 * s : 512 * (s + 1)],
                            ab_s[:, k * OH : (k + 1) * OH],
                            xts[(s, k)][:],
                            start=(k == 0),
                            stop=(k == 3),
                        )
                    if s == 1:
                        emit_taps(0, JB)
                emit_taps(JB, OW)

                nc.scalar.dma_start(
                    out=out[n].rearrange("i j c -> i (j c)"), in_=ot[:]
                )

            def emit_all():
                for n in range(N_PER_CORE):
                    # tail_split = how many trailing images get the
                    # W-thirds streaming treatment
                    if dma_only or n < N_PER_CORE - tail_split:
                        emit_image(n)
                    else:
                        emit_image_tailsplit(n)

            if repeats == 1:
                emit_all()
            else:
                with tc.For_i(0, repeats, 1):
                    emit_all()

    nc.finalize()
    return nc


_NC_CACHE = None


def _get_nc() -> bass.Bass:
    global _NC_CACHE
    if _NC_CACHE is None:
        _NC_CACHE = build_nc()
    return _NC_CACHE


def run(x: np.ndarray, trace: bool = False):
    """Returns (out [32,128,128,3] f32, exec_time_ns or None)."""
    x = np.ascontiguousarray(np.asarray(x, dtype=np.float32))
    assert x.shape == (N_CORES * N_PER_CORE, H, W, C), x.shape
    ab = _band_matrix()
    nc = _get_nc()
    in_maps = [
        {"x": x[i * N_PER_CORE : (i + 1) * N_PER_CORE], "ab": ab}
        for i in range(N_CORES)
    ]
    res = run_bass_kernel_spmd(nc, in_maps, core_ids=list(range(N_CORES)), trace=trace)
    outs = [np.asarray(res.results[i]["out"]) for i in range(N_CORES)]
    return np.concatenate(outs, axis=0), res.exec_time_ns


def kernel(x: np.ndarray) -> np.ndarray:
    out, _ = run(x, trace=False)
    return out

